# revision 31
# baseline (speedup 1.0000x reference)
"""Distributed multi-head attention for trn2 (8 NeuronCores).

Problem: B=4, S=1024, H=1024, nh=16, hd=64; mask all-ones, biases zero
(fixed by the input spec), so neither reaches the device.

Sharding: core c = b*2 + g handles batch b = c//2 and head-group g = c%2
(8 heads = 512 hidden dims).  Per core:

  qT/kT  : bf16 matmuls; ACT copies psum->bf16 pair tiles (tile m holds
           head 2m's 64 dims on partitions 0-63, head 2m+1 on 64-127).
  v      : fp8 e4m3 DoubleRow matmuls (x fp8 x Wv*256 fp8, K=256/instr)
           - the only fp8 path: softmax's positive weights average the
           v-side quantization noise down by ~sqrt(S), so it is free.
  scores : per head pair, two row-tiled K=64 bf16 matmuls run
           concurrently in the PE array (tile_position (0,0)/(64,0)),
           writing both heads' [tk, tq-half] tiles into separate psum
           banks of one [128,1024] tile.
  exp    : split between ACT (exact, scale=1/8) and DVE (Schraudolph
           bit-trick exp -> bf16 bits via int16 tensor_scalar), since
           the 8.4M activations/core otherwise bound the kernel.
  ctx    : bf16; probsT streaming over v|ones stationary; denominators
           land on psum rows 64-127; DVE reciprocal + multiply -> ctxT.
  out    : bf16, ACT/DVE copies, per-chunk DMA of the [1024,1024] f32
           partial.

The emission is software-pipelined at head-pair granularity: the qk
projection of pair m+1 and the ctx/finalize of group g-1 interleave
with the scores/exp slots of group g (group = (pair, tq-half)), so PE
work hides the ACT/DVE softmax cost.  Host sums the two partials of
each batch (row-parallel Wo unshard) and stacks the 4 batches.
"""

import sys

import numpy as np

sys.path.insert(0, "/opt/trn_rl_repo")

import ml_dtypes  # noqa: E402

import concourse.tile as tile  # noqa: E402
from concourse import bacc, mybir  # noqa: E402
from concourse.bass_utils import run_bass_kernel_spmd  # noqa: E402

S = 1024
H = 1024
NH_LOC = 8
HD = 64
HG = 512
P = 128

F32 = mybir.dt.float32
BF16 = mybir.dt.bfloat16
FP8 = mybir.dt.float8e4
I16 = mybir.dt.int16
DR = mybir.MatmulPerfMode.DoubleRow
Copy = mybir.ActivationFunctionType.Copy
Exp = mybir.ActivationFunctionType.Exp

SQ = 256.0  # host premultiplier on Wv before fp8 cast
# Schraudolph exp(x/8) -> bf16 bits: i16 = x*A + B
SCH_A = 0.125 * np.log2(np.e) * 128.0
SCH_B = 127.0 * 128.0 - 7.41
DVE_TKS = {0: (), 1: ()}  # per-th tk slots whose exp runs on DVE

_CACHE: dict = {}


def _build_graph(reps: int = 1, timing: bool = False, phases=("qkv", "attn", "exp", "ctx", "out")):
    nc = bacc.Bacc("TRN2", target_bir_lowering=False, debug=False, num_devices=8)

    kind = "Internal" if timing else "ExternalInput"
    okind = "Internal" if timing else "ExternalOutput"
    xt_d = nc.dram_tensor("xt", [P, 8, S], BF16, kind=kind).ap()
    wqt_d = nc.dram_tensor("wqt", [P, 8, HG], BF16, kind=kind).ap()
    wkt_d = nc.dram_tensor("wkt", [P, 8, HG], BF16, kind=kind).ap()
    wvt_d = nc.dram_tensor("wvt", [P, 8, HG], BF16, kind=kind).ap()
    wot_d = nc.dram_tensor("wot", [P, 4, H], BF16, kind=kind).ap()
    out_d = nc.dram_tensor("out_p", [S, H], F32, kind=okind).ap()
    tok_d = (
        nc.dram_tensor("tok", [1, 4], F32, kind="ExternalOutput").ap()
        if timing
        else None
    )

    with tile.TileContext(nc) as tc:
        with tc.tile_pool(name="inp", bufs=1) as inp:
            T = _persistent(tc, inp, phases)
            _dma_inputs(tc, T, xt_d, wqt_d, wkt_d, wvt_d, wot_d)
            if reps == 1:
                _compute(tc, T, out_d, tok_d, phases)
            else:
                hints = (
                    mybir.EngineType.PE,
                    mybir.EngineType.DVE,
                    mybir.EngineType.Pool,
                )
                with tc.For_i(0, reps, 1, hint_engines=hints):
                    _compute(tc, T, out_d, tok_d, phases)

    nc.compile()
    return nc


def _persistent(tc, inp, phases):
    nc = tc.nc
    T = {}
    T["xt"] = inp.tile([P, 8, S], BF16, tag="xt", name="xt")
    T["wqt"] = inp.tile([P, 8, HG], BF16, tag="wqt", name="wqt")
    T["wkt"] = inp.tile([P, 8, HG], BF16, tag="wkt", name="wkt")
    T["wvt"] = inp.tile([P, 8, HG], BF16, tag="wvt", name="wvt")
    T["wot"] = inp.tile([P, 4, H], BF16, tag="wot", name="wot")
    T["qT"] = [inp.tile([P, S], BF16, tag=f"qT{m}", name=f"qT{m}") for m in range(4)]
    T["kT"] = [inp.tile([P, S], BF16, tag=f"kT{h}", name=f"kT{h}") for h in range(8)]
    T["v"] = [
        inp.tile([P, NH_LOC, P], BF16, tag=f"v{i}", name=f"v{i}") for i in range(8)
    ]
    T["ctxT"] = inp.tile([P, 4, S], BF16, tag="ctxT", name="ctxT")
    T["oa"] = [inp.tile([P, H], F32, tag=f"oa{i}", name=f"oa{i}") for i in range(8)]
    for i in range(8):
        nc.gpsimd.memset(T["v"][i][:, :, HD:P], 1.0)
    for h in range(8):
        pad = slice(HD, P) if h % 2 == 0 else slice(0, HD)
        nc.gpsimd.memset(T["kT"][h][pad, :], 0.0)
    # pre-seed tiles for phase-subset timing runs
    if "qkv" not in phases:
        if "attn" in phases:
            for m in range(4):
                nc.gpsimd.memset(T["qT"][m][:], 0.125)
            for h in range(8):
                nc.gpsimd.memset(T["kT"][h][:], 0.125)
        if "ctx" in phases:
            for i in range(8):
                nc.gpsimd.memset(T["v"][i][:, :, 0:HD], 0.125)
    if "out" in phases and "ctx" not in phases:
        nc.gpsimd.memset(T["ctxT"][:, :, :], 0.125)
    return T


def _dma_inputs(tc, T, xt_d, wqt_d, wkt_d, wvt_d, wot_d):
    nc = tc.nc
    nc.sync.dma_start(T["xt"][:, :, :], xt_d)
    nc.sync.dma_start(T["wqt"][:, :, :], wqt_d)
    nc.sync.dma_start(T["wkt"][:, :, :], wkt_d)
    nc.sync.dma_start(T["wvt"][:, :, :], wvt_d)
    nc.sync.dma_start(T["wot"][:, :, :], wot_d)


def _compute(tc, T, out_d, tok_d=None, phases=("qkv", "attn", "exp", "ctx", "out")):
    nc = tc.nc
    from contextlib import ExitStack

    do_qkv = "qkv" in phases
    do_attn = "attn" in phases
    do_exp = "exp" in phases
    do_ctx = "ctx" in phases
    do_out = "out" in phases

    ctx = ExitStack()
    with ctx:
        ps_a = ctx.enter_context(tc.tile_pool(name="ps_a", bufs=2, space="PSUM"))
        ps_b = ctx.enter_context(tc.tile_pool(name="ps_b", bufs=4, space="PSUM"))
        probs_pool = ctx.enter_context(tc.tile_pool(name="probs", bufs=20))
        probs16_pool = ctx.enter_context(tc.tile_pool(name="probs16", bufs=10))
        small = ctx.enter_context(tc.tile_pool(name="small", bufs=4))

        def emit_qk(m):
            # bf16 projections for head pair m -> qT[m] (pair tile) and
            # kT[2m]/kT[2m+1] (zero-padded per-head tiles at pair rows).
            # psum [128,1024] from ps_a (idle during qkv) keeps ps_b free
            # for the long-lived ctx chains.
            for wt, is_q in ((T["wqt"], True), (T["wkt"], False)):
                ps = ps_a.tile([P, S], F32, tag="ps", name="ps_qk")
                for th in range(2):
                    for kc in range(8):
                        nc.tensor.matmul(
                            ps[:, th * HG : (th + 1) * HG],
                            wt[:, kc, m * P : (m + 1) * P],
                            T["xt"][:, kc, th * HG : (th + 1) * HG],
                            start=(kc == 0),
                            stop=(kc == 7),
                        )
                if is_q:
                    nc.vector.tensor_copy(T["qT"][m][:, :], ps[:])
                else:
                    nc.vector.tensor_copy(T["kT"][2 * m][0:HD, :], ps[0:HD, :])
                    nc.vector.tensor_copy(
                        T["kT"][2 * m + 1][HD:P, :], ps[HD:P, :]
                    )

        def emit_v():
            for tcv in range(8):
                psv = ps_b.tile([P, NH_LOC, HD], F32, tag="ps", name="ps_v")
                for kc in range(8):
                    nc.tensor.matmul(
                        psv[:, :, :],
                        T["xt"][:, kc, tcv * P : (tcv + 1) * P],
                        T["wvt"][:, kc, :],
                        start=(kc == 0),
                        stop=(kc == 7),
                    )
                nc.vector.tensor_copy(T["v"][tcv][:, :, 0:HD], psv[:, :, :])

        def emit_slot_scores(m, th, tk):
            ps = ps_a.tile([P, S], F32, tag="ps", name="ps_s")
            nc.tensor.matmul(
                ps[:, 0:HG],
                T["kT"][2 * m][:, tk * P : (tk + 1) * P],
                T["qT"][m][:, th * HG : (th + 1) * HG],
                start=True,
                stop=True,
            )
            nc.tensor.matmul(
                ps[:, HG:S],
                T["kT"][2 * m + 1][:, tk * P : (tk + 1) * P],
                T["qT"][m][:, th * HG : (th + 1) * HG],
                start=True,
                stop=True,
            )

            pb = probs_pool.tile([P, S], BF16, tag="pb", name="pb")
            if do_exp:
                nc.scalar.activation(pb[:], ps[:], Exp, scale=0.125)
            return pb

        def emit_ctx_mm(pm, pth, par, tk, ps_c, probs):
            h = 2 * pm + par
            nc.tensor.matmul(
                ps_c[:],
                T["v"][tk][:, h, :],
                probs[tk][:, par * HG : (par + 1) * HG],
                start=(tk == 0),
                stop=(tk == 7),
            )

        def emit_fin(pm, pth, par, ps_c):
            rp = small.tile([HD, HG], F32, tag="rp", name="rp")
            nc.vector.reciprocal(rp[:], ps_c[HD:P, :])
            nc.vector.tensor_tensor(
                T["ctxT"][par * HD : (par + 1) * HD, pm, pth * HG : (pth + 1) * HG],
                ps_c[0:HD, :],
                rp[:],
                mybir.AluOpType.mult,
            )

        # ---- software-pipelined qkv + attention ------------------------
        if do_qkv:
            for m in range(4):
                emit_qk(m)
        prev = None  # (pm, pth, probs-by-tk) of the previous group
        if do_qkv and not do_attn:
            emit_v()
        for g in range(8):
            if not do_attn:
                break
            m, th = g // 2, g % 2
            ps_cc = None
            if do_ctx and prev is not None:
                ps_cc = [
                    ps_b.tile([P, HG], F32, tag="ps", name="ps_cc") for _ in range(2)
                ]
            if do_qkv and g == 1:
                emit_v()
            # final group: its own ctx chains interleave at slot level so
            # the tail is just fins + out
            ps_cl = None
            if do_ctx and g == 7:
                ps_cl = [
                    ps_b.tile([P, HG], F32, tag="ps", name="ps_cl")
                    for _ in range(2)
                ]
            cur = {}
            for tk in range(8):
                if ps_cc is not None:
                    for par in range(2):
                        emit_ctx_mm(prev[0], prev[1], par, tk, ps_cc[par], prev[2])
                cur[tk] = emit_slot_scores(m, th, tk)
                if ps_cl is not None:
                    for par in range(2):
                        emit_ctx_mm(m, th, par, tk, ps_cl[par], cur)
            if ps_cc is not None:
                for par in range(2):
                    emit_fin(prev[0], prev[1], par, ps_cc[par])
            if ps_cl is not None:
                for par in range(2):
                    emit_fin(m, th, par, ps_cl[par])
            prev = (m, th, cur)

        # ---- output projection (bf16) ---------------------------------
        if do_out:
            for tcv in range(8):
                for ho in range(2):
                    pso = ps_b.tile([P, HG], F32, tag="ps", name="ps_o")
                    for a in range(4):
                        nc.tensor.matmul(
                            pso[:],
                            T["ctxT"][:, a, tcv * P : (tcv + 1) * P],
                            T["wot"][:, a, ho * HG : (ho + 1) * HG],
                            start=(a == 0),
                            stop=(a == 3),
                        )
                    nc.vector.tensor_copy(
                        T["oa"][tcv][:, ho * HG : (ho + 1) * HG], pso[:]
                    )
                nc.sync.dma_start(
                    out_d[tcv * P : (tcv + 1) * P, :], T["oa"][tcv][:]
                )

        if tok_d is not None:
            tk_t = small.tile([1, 4], F32, tag="tok", name="tok")
            nc.gpsimd.memset(tk_t[:], 0.0)
            nc.sync.dma_start(tok_d[:], tk_t[:])


def _get_nc():
    if "nc" not in _CACHE:
        _CACHE["nc"] = _build_graph()
    return _CACHE["nc"]


def _to_blocks(a, nblk, dtype):
    """[rows, cols] -> [128, nblk, cols] with row = blk*128 + p."""
    rows, cols = a.shape
    assert rows == nblk * P
    if dtype == ml_dtypes.float8_e4m3:
        a = np.clip(a, -240.0, 240.0)
    return np.ascontiguousarray(
        a.reshape(nblk, P, cols).transpose(1, 0, 2).astype(dtype)
    )


def kernel(x, mask, Wq, bq, Wk, bk, Wv, bv, Wo, bo):
    x = np.asarray(x, dtype=np.float32)
    Wq = np.asarray(Wq, dtype=np.float32)
    Wk = np.asarray(Wk, dtype=np.float32)
    Wv = np.asarray(Wv, dtype=np.float32)
    Wo = np.asarray(Wo, dtype=np.float32)

    nc = _get_nc()
    bf = ml_dtypes.bfloat16
    f8 = ml_dtypes.float8_e4m3
    in_maps = []
    for c in range(8):
        b, g = c // 2, c % 2
        sl = slice(g * HG, (g + 1) * HG)
        in_maps.append(
            {
                "xt": _to_blocks(x[b].T, 8, bf),
                "wqt": _to_blocks(Wq[sl, :].T, 8, bf),
                "wkt": _to_blocks(Wk[sl, :].T, 8, bf),
                "wvt": _to_blocks(Wv[sl, :].T, 8, bf),
                "wot": _to_blocks(Wo[:, sl].T, 4, bf),
            }
        )
    res = run_bass_kernel_spmd(
        nc, in_maps, core_ids=list(range(8)), **_CACHE.get("run_kwargs", {})
    )
    _CACHE["last_result"] = res
    outs = [res.results[c]["out_p"] for c in range(8)]
    return np.stack(
        [outs[2 * b] + outs[2 * b + 1] for b in range(4)]
    ).astype(np.float32)


# revision 35
# speedup vs baseline: 1.2798x; 1.2798x over previous
"""Distributed multi-head attention for trn2 (8 NeuronCores).

Problem: B=4, S=1024, H=1024, nh=16, hd=64; mask all-ones, biases zero
(fixed by the input spec), so neither reaches the device.

Sharding: core c = b*2 + g handles batch b = c//2 and head-group g = c%2
(8 heads = 512 hidden dims).  Per core:

  qT/kT  : bf16 matmuls; ACT copies psum->bf16 pair tiles (tile m holds
           head 2m's 64 dims on partitions 0-63, head 2m+1 on 64-127).
  v      : fp8 e4m3 DoubleRow matmuls (x fp8 x Wv*256 fp8, K=256/instr)
           - the only fp8 path: softmax's positive weights average the
           v-side quantization noise down by ~sqrt(S), so it is free.
  scores : per head pair, two row-tiled K=64 bf16 matmuls run
           concurrently in the PE array (tile_position (0,0)/(64,0)),
           writing both heads' [tk, tq-half] tiles into separate psum
           banks of one [128,1024] tile.
  exp    : split between ACT (exact, scale=1/8) and DVE (Schraudolph
           bit-trick exp -> bf16 bits via int16 tensor_scalar), since
           the 8.4M activations/core otherwise bound the kernel.
  ctx    : bf16; probsT streaming over v|ones stationary; denominators
           land on psum rows 64-127; DVE reciprocal + multiply -> ctxT.
  out    : bf16, ACT/DVE copies, per-chunk DMA of the [1024,1024] f32
           partial.

The emission is software-pipelined at head-pair granularity: the qk
projection of pair m+1 and the ctx/finalize of group g-1 interleave
with the scores/exp slots of group g (group = (pair, tq-half)), so PE
work hides the ACT/DVE softmax cost.  Host sums the two partials of
each batch (row-parallel Wo unshard) and stacks the 4 batches.
"""

import sys

import numpy as np

sys.path.insert(0, "/opt/trn_rl_repo")

import ml_dtypes  # noqa: E402

import concourse.tile as tile  # noqa: E402
from concourse import bacc, mybir  # noqa: E402
from concourse.bass_utils import run_bass_kernel_spmd  # noqa: E402

S = 1024
H = 1024
NH_LOC = 8
HD = 64
HG = 512
P = 128

F32 = mybir.dt.float32
BF16 = mybir.dt.bfloat16
FP8 = mybir.dt.float8e4
I16 = mybir.dt.int16
DR = mybir.MatmulPerfMode.DoubleRow
Copy = mybir.ActivationFunctionType.Copy
Exp = mybir.ActivationFunctionType.Exp

SQ = 256.0  # host premultiplier on Wv before fp8 cast
# Schraudolph exp(x/8) -> bf16 bits: i16 = x*A + B
SCH_A = 0.125 * np.log2(np.e) * 128.0
SCH_B = 127.0 * 128.0 - 7.41
DVE_TKS = {0: (), 1: ()}  # per-th tk slots whose exp runs on DVE

_CACHE: dict = {}


def _build_graph(reps: int = 1, timing: bool = False, phases=("qkv", "attn", "exp", "ctx", "out")):
    nc = bacc.Bacc("TRN2", target_bir_lowering=False, debug=False, num_devices=8)

    kind = "Internal" if timing else "ExternalInput"
    okind = "Internal" if timing else "ExternalOutput"
    xt_d = nc.dram_tensor("xt", [P, 8, S], BF16, kind=kind).ap()
    wqt_d = nc.dram_tensor("wqt", [P, 8, HG], BF16, kind=kind).ap()
    wkt_d = nc.dram_tensor("wkt", [P, 8, HG], BF16, kind=kind).ap()
    wvt_d = nc.dram_tensor("wvt", [P, 8, HG], BF16, kind=kind).ap()
    wot_d = nc.dram_tensor("wot", [P, 4, H], BF16, kind=kind).ap()
    out_d = nc.dram_tensor("out_p", [S, H], F32, kind=okind).ap()
    tok_d = (
        nc.dram_tensor("tok", [1, 4], F32, kind="ExternalOutput").ap()
        if timing
        else None
    )

    with tile.TileContext(nc) as tc:
        with tc.tile_pool(name="inp", bufs=1) as inp:
            T = _persistent(tc, inp, phases)
            _dma_inputs(tc, T, xt_d, wqt_d, wkt_d, wvt_d, wot_d)
            if reps == 1:
                _compute(tc, T, out_d, tok_d, phases)
            else:
                hints = (
                    mybir.EngineType.PE,
                    mybir.EngineType.DVE,
                    mybir.EngineType.Pool,
                )
                with tc.For_i(0, reps, 1, hint_engines=hints):
                    _compute(tc, T, out_d, tok_d, phases)

    nc.compile()
    return nc


def _persistent(tc, inp, phases):
    nc = tc.nc
    T = {}
    T["xt"] = inp.tile([P, 8, S], BF16, tag="xt", name="xt")
    T["wqt"] = inp.tile([P, 8, HG], BF16, tag="wqt", name="wqt")
    T["wkt"] = inp.tile([P, 8, HG], BF16, tag="wkt", name="wkt")
    T["wvt"] = inp.tile([P, 8, HG], BF16, tag="wvt", name="wvt")
    T["wot"] = inp.tile([P, 4, H], BF16, tag="wot", name="wot")
    T["qT"] = [inp.tile([P, S], BF16, tag=f"qT{m}", name=f"qT{m}") for m in range(4)]
    T["kT"] = [inp.tile([P, S], BF16, tag=f"kT{h}", name=f"kT{h}") for h in range(8)]
    T["v"] = [
        inp.tile([P, NH_LOC, P], BF16, tag=f"v{i}", name=f"v{i}") for i in range(8)
    ]
    T["ctxT"] = inp.tile([P, 4, S], BF16, tag="ctxT", name="ctxT")
    T["oa"] = [inp.tile([P, H], F32, tag=f"oa{i}", name=f"oa{i}") for i in range(8)]
    for i in range(8):
        nc.gpsimd.memset(T["v"][i][:, :, HD:P], 1.0)
    for h in range(8):
        pad = slice(HD, P) if h % 2 == 0 else slice(0, HD)
        nc.gpsimd.memset(T["kT"][h][pad, :], 0.0)
    # pre-seed tiles for phase-subset timing runs
    if "qkv" not in phases:
        if "attn" in phases:
            for m in range(4):
                nc.gpsimd.memset(T["qT"][m][:], 0.125)
            for h in range(8):
                nc.gpsimd.memset(T["kT"][h][:], 0.125)
        if "ctx" in phases:
            for i in range(8):
                nc.gpsimd.memset(T["v"][i][:, :, 0:HD], 0.125)
    if "out" in phases and "ctx" not in phases:
        nc.gpsimd.memset(T["ctxT"][:, :, :], 0.125)
    return T


def _dma_inputs(tc, T, xt_d, wqt_d, wkt_d, wvt_d, wot_d):
    nc = tc.nc
    nc.sync.dma_start(T["xt"][:, :, :], xt_d)
    nc.sync.dma_start(T["wqt"][:, :, :], wqt_d)
    nc.sync.dma_start(T["wkt"][:, :, :], wkt_d)
    nc.sync.dma_start(T["wvt"][:, :, :], wvt_d)
    nc.sync.dma_start(T["wot"][:, :, :], wot_d)


def _compute(tc, T, out_d, tok_d=None, phases=("qkv", "attn", "exp", "ctx", "out")):
    nc = tc.nc
    from contextlib import ExitStack

    do_qkv = "qkv" in phases
    do_attn = "attn" in phases
    do_exp = "exp" in phases
    do_ctx = "ctx" in phases
    do_out = "out" in phases

    ctx = ExitStack()
    with ctx:
        ps_a = ctx.enter_context(tc.tile_pool(name="ps_a", bufs=2, space="PSUM"))
        ps_b = ctx.enter_context(tc.tile_pool(name="ps_b", bufs=4, space="PSUM"))
        probs_pool = ctx.enter_context(tc.tile_pool(name="probs", bufs=20))
        probs16_pool = ctx.enter_context(tc.tile_pool(name="probs16", bufs=10))
        small = ctx.enter_context(tc.tile_pool(name="small", bufs=4))

        def emit_qk(m):
            # bf16 projections for head pair m -> qT[m] (pair tile) and
            # kT[2m]/kT[2m+1] (zero-padded per-head tiles at pair rows).
            # psum [128,1024] from ps_a (idle during qkv) keeps ps_b free
            # for the long-lived ctx chains.
            for wt, is_q in ((T["wqt"], True), (T["wkt"], False)):
                ps = ps_a.tile([P, S], F32, tag="ps", name="ps_qk")
                for th in range(2):
                    for kc in range(8):
                        nc.tensor.matmul(
                            ps[:, th * HG : (th + 1) * HG],
                            wt[:, kc, m * P : (m + 1) * P],
                            T["xt"][:, kc, th * HG : (th + 1) * HG],
                            start=(kc == 0),
                            stop=(kc == 7),
                        )
                if is_q:
                    nc.vector.tensor_copy(T["qT"][m][:, :], ps[:])
                else:
                    nc.vector.tensor_copy(T["kT"][2 * m][0:HD, :], ps[0:HD, :])
                    nc.vector.tensor_copy(
                        T["kT"][2 * m + 1][HD:P, :], ps[HD:P, :]
                    )

        def emit_v():
            for tcv in range(8):
                psv = ps_b.tile([P, NH_LOC, HD], F32, tag="ps", name="ps_v")
                for kc in range(8):
                    nc.tensor.matmul(
                        psv[:, :, :],
                        T["xt"][:, kc, tcv * P : (tcv + 1) * P],
                        T["wvt"][:, kc, :],
                        start=(kc == 0),
                        stop=(kc == 7),
                    )
                nc.vector.tensor_copy(T["v"][tcv][:, :, 0:HD], psv[:, :, :])

        def emit_slot_scores(m, th, tk):
            ps = ps_a.tile([P, S], F32, tag="ps", name="ps_s")
            nc.tensor.matmul(
                ps[:, 0:HG],
                T["kT"][2 * m][:, tk * P : (tk + 1) * P],
                T["qT"][m][:, th * HG : (th + 1) * HG],
                start=True,
                stop=True,
            )
            nc.tensor.matmul(
                ps[:, HG:S],
                T["kT"][2 * m + 1][:, tk * P : (tk + 1) * P],
                T["qT"][m][:, th * HG : (th + 1) * HG],
                start=True,
                stop=True,
            )

            pb = probs_pool.tile([P, S], BF16, tag="pb", name="pb")
            if do_exp:
                nc.scalar.activation(pb[:], ps[:], Exp, scale=0.125)
            return pb

        def emit_ctx_mm(pm, pth, par, tk, ps_c, probs):
            h = 2 * pm + par
            nc.tensor.matmul(
                ps_c[:],
                T["v"][tk][:, h, :],
                probs[tk][:, par * HG : (par + 1) * HG],
                start=(tk == 0),
                stop=(tk == 7),
            )

        def emit_fin(pm, pth, par, ps_c):
            dn = small.tile([HD, HG], F32, tag="dn", name="dn")
            rp = small.tile([HD, HG], F32, tag="rp", name="rp")
            nc.vector.tensor_copy(dn[:], ps_c[HD:P, :])
            nc.vector.reciprocal_approx_fast(rp[:], dn[:])
            nc.vector.tensor_tensor(
                T["ctxT"][par * HD : (par + 1) * HD, pm, pth * HG : (pth + 1) * HG],
                ps_c[0:HD, :],
                rp[:],
                mybir.AluOpType.mult,
            )

        # ---- software-pipelined qkv + attention ------------------------
        if do_qkv:
            for m in range(4):
                emit_qk(m)
        prev = None  # (pm, pth, probs-by-tk) of the previous group
        if do_qkv and not do_attn:
            emit_v()
        for g in range(8):
            if not do_attn:
                break
            m, th = g // 2, g % 2
            ps_cc = None
            if do_ctx and prev is not None:
                ps_cc = [
                    ps_b.tile([P, HG], F32, tag="ps", name="ps_cc") for _ in range(2)
                ]
            if do_qkv and g == 1:
                emit_v()
            # final group: its own ctx chains interleave at slot level so
            # the tail is just fins + out
            ps_cl = None
            if do_ctx and g == 7:
                ps_cl = [
                    ps_b.tile([P, HG], F32, tag="ps", name="ps_cl")
                    for _ in range(2)
                ]
            cur = {}
            for tk in range(8):
                if ps_cc is not None:
                    for par in range(2):
                        emit_ctx_mm(prev[0], prev[1], par, tk, ps_cc[par], prev[2])
                cur[tk] = emit_slot_scores(m, th, tk)
                if ps_cl is not None:
                    for par in range(2):
                        emit_ctx_mm(m, th, par, tk, ps_cl[par], cur)
            if ps_cc is not None:
                for par in range(2):
                    emit_fin(prev[0], prev[1], par, ps_cc[par])
            if ps_cl is not None:
                for par in range(2):
                    emit_fin(m, th, par, ps_cl[par])
            prev = (m, th, cur)

        # ---- output projection (bf16) ---------------------------------
        if do_out:
            for tcv in range(8):
                for ho in range(2):
                    pso = ps_b.tile([P, HG], F32, tag="ps", name="ps_o")
                    for a in range(4):
                        nc.tensor.matmul(
                            pso[:],
                            T["ctxT"][:, a, tcv * P : (tcv + 1) * P],
                            T["wot"][:, a, ho * HG : (ho + 1) * HG],
                            start=(a == 0),
                            stop=(a == 3),
                        )
                    nc.vector.tensor_copy(
                        T["oa"][tcv][:, ho * HG : (ho + 1) * HG], pso[:]
                    )
                nc.sync.dma_start(
                    out_d[tcv * P : (tcv + 1) * P, :], T["oa"][tcv][:]
                )

        if tok_d is not None:
            tk_t = small.tile([1, 4], F32, tag="tok", name="tok")
            nc.gpsimd.memset(tk_t[:], 0.0)
            nc.sync.dma_start(tok_d[:], tk_t[:])


def _get_nc():
    if "nc" not in _CACHE:
        _CACHE["nc"] = _build_graph()
    return _CACHE["nc"]


def _to_blocks(a, nblk, dtype):
    """[rows, cols] -> [128, nblk, cols] with row = blk*128 + p."""
    rows, cols = a.shape
    assert rows == nblk * P
    if dtype == ml_dtypes.float8_e4m3:
        a = np.clip(a, -240.0, 240.0)
    return np.ascontiguousarray(
        a.reshape(nblk, P, cols).transpose(1, 0, 2).astype(dtype)
    )


def kernel(x, mask, Wq, bq, Wk, bk, Wv, bv, Wo, bo):
    x = np.asarray(x, dtype=np.float32)
    Wq = np.asarray(Wq, dtype=np.float32)
    Wk = np.asarray(Wk, dtype=np.float32)
    Wv = np.asarray(Wv, dtype=np.float32)
    Wo = np.asarray(Wo, dtype=np.float32)

    nc = _get_nc()
    bf = ml_dtypes.bfloat16
    f8 = ml_dtypes.float8_e4m3
    in_maps = []
    for c in range(8):
        b, g = c // 2, c % 2
        sl = slice(g * HG, (g + 1) * HG)
        in_maps.append(
            {
                "xt": _to_blocks(x[b].T, 8, bf),
                "wqt": _to_blocks(Wq[sl, :].T, 8, bf),
                "wkt": _to_blocks(Wk[sl, :].T, 8, bf),
                "wvt": _to_blocks(Wv[sl, :].T, 8, bf),
                "wot": _to_blocks(Wo[:, sl].T, 4, bf),
            }
        )
    res = run_bass_kernel_spmd(
        nc, in_maps, core_ids=list(range(8)), **_CACHE.get("run_kwargs", {})
    )
    _CACHE["last_result"] = res
    outs = [res.results[c]["out_p"] for c in range(8)]
    return np.stack(
        [outs[2 * b] + outs[2 * b + 1] for b in range(4)]
    ).astype(np.float32)
